# revision 1
# baseline (speedup 1.0000x reference)
"""DiT block kernel for TRN2, 8 NeuronCores.

Sharding: DP=4 over batch x TP=2 over heads (Megatron style).
Core c: batch b=c//2, half hf=c%2 (8 of 16 heads, 2048/4096 MLP cols, 512/1024
rows of the row-parallel weights).

Device layout is feature-major: activations [feature(partitions), token(free)], so
every matmul chains without transposes: outT = W.T @ actT with lhsT=W natural layout.
v^T (lhsT of attn@v) is produced by swapping matmul operands. Softmax runs without
max-subtraction (scores are O(1)); denominators come from an augmented ones-column in
v^T (row 64 of the [65, tok] attention output).

Dtypes: trunk matmuls in float32r (TF32-like, 1 cyc/row, rel err ~1.5e-4); the
attention block (q/k/v/P/attn-out) and proj/out/fc2 weights in bf16. PSUM fp32.

Comms: 3 pair AllReduces (adaLN mod, self-attn proj+residual, cross-attn out+residual);
the fc2 reduce is folded into the output (host adds the two partials).

Weights stream through SBUF in M-blocks: slot [128, kc, mblk], one DMA per k-slice.
"""
import sys
import numpy as np

sys.path.insert(0, "/opt/trn_rl_repo")

import ml_dtypes
import concourse.bass as bass
import concourse.mybir as mybir
import concourse.tile as tile
from concourse import bacc
from concourse.bass_utils import run_bass_kernel_spmd

FP32 = mybir.dt.float32
FP32R = mybir.dt.float32r
BF16 = mybir.dt.bfloat16
AF = mybir.ActivationFunctionType
ALU = mybir.AluOpType

B, N, D, H, TD, TL = 4, 1024, 1024, 16, 768, 77
HD = 64
EPS = 1e-6
HL = 8
DL = 512
FFL = 2048
T = 1024
TLP = 80    # ctx tokens padded to even (fp32r ISA: innermost counts must be even)
NCH = D // 128
REPLICA_GROUPS = [[0, 1], [2, 3], [4, 5], [6, 7]]

# SBUF knobs (KB/partition noted at default)
P_BUFS = 7        # exp(P) tiles  (bf16 [128,1024] = 2KB each)
WST_BUFS = 2      # fp32r weight blocks (8KB each)
WST2_BUFS = 2     # bf16 weight blocks (8KB each)
XSQ_BUFS = 1
SCR_BUFS = 2


def _declare(nc):
    d = {}

    def inp(name, shape, dt):
        d[name] = nc.dram_tensor(name, list(shape), dt, kind="ExternalInput").ap()

    inp("xT", (D, T), FP32R)
    inp("cT", (128, 4), FP32)
    inp("teT", (TD, TLP), FP32R)
    inp("w_ada", (12, 128, 4, 512), FP32R)
    inp("b_ada", (1, 6 * D), FP32R)
    inp("w_qkv", (6, 128, 8, 256), FP32R)
    inp("b_qkT", (128, 8), FP32)
    inp("b_v", (1, DL), FP32)
    inp("w_proj", (2, 128, 4, 512), BF16)
    inp("b_projT", (128, 8), FP32)
    inp("w_ctx", (4, 128, 6, 256), FP32R)
    inp("b_ctxT", (128, 8), FP32)
    inp("w_q", (2, 128, 8, 256), FP32R)
    inp("w_k", (2, 128, 8, 256), FP32R)
    inp("w_v", (2, 128, 8, 256), FP32R)
    inp("w_out", (2, 128, 4, 512), BF16)
    inp("b_outT", (128, 8), FP32)
    inp("w_fc1", (8, 128, 8, 256), FP32R)
    inp("b_fc1T", (128, 16), FP32)
    inp("w_fc2", (4, 128, 16, 256), BF16)
    inp("b_fc2T", (128, 8), FP32)
    inp("ones_r", (128, 128), FP32R)
    inp("ones_b", (128, 8), BF16)
    inp("maskT", (128, 1), FP32)
    d["out_xT"] = nc.dram_tensor("out_xT", [D, T], FP32R, kind="ExternalOutput").ap()
    return d


def _emit(tc, io, pools, nocc=False):
    nc = tc.nc
    sb = pools["sb"]
    xp, hp, qkp, vtp, atp, pp = (pools[k] for k in ("xp", "hp", "qkp", "vtp", "atp", "pp"))
    wst, wst2, hidp, xsqp, scr, rcpp = (pools[k] for k in
                                        ("wst", "wst2", "hidp", "xsqp", "scr", "rcpp"))
    vecp = pools["vecp"]
    ps_mm, ps_aux = pools["ps_mm"], pools["ps_aux"]
    dram = pools["dram"]

    ones = sb.tile([128, 128], FP32R, tag="ones")
    nc.sync.dma_start(out=ones, in_=io["ones_r"])
    ones_b = sb.tile([128, 8], BF16, tag="ones_b")
    nc.sync.dma_start(out=ones_b, in_=io["ones_b"])
    mask = sb.tile([128, 1], FP32, tag="mask")
    nc.sync.dma_start(out=mask, in_=io["maskT"])

    def load_wblock(src_ap, dt, bi, pool):
        """Load pre-tiled weight block bi: host layout [nb, 128, kc, mblk] ->
        one fully-contiguous-per-partition DMA."""
        _, _, kc, mblk = src_ap.shape
        wt = pool.tile([128, kc, mblk], dt, tag="w")
        nc.sync.dma_start(out=wt, in_=src_ap[bi])
        return wt

    # ---------------- Stage 0: adaLN ----------------
    cT = sb.tile([128, 4], FP32, tag="cT")
    nc.sync.dma_start(out=cT, in_=io["cT"])
    cs = sb.tile([128, 4], FP32R, tag="cs")
    nc.scalar.activation(cs, cT, AF.Silu)

    ar_mod_in = dram.tile([1, 6 * D], FP32, tag="armod_i")
    ar_mod_out = dram.tile([1, 6 * D], FP32, tag="armod_o")

    for mb in range(12):   # w_ada rhs-blocks [512, 512]
        wt = load_wblock(io["w_ada"], FP32R, mb, wst)
        b_sl = sb.tile([1, 512], FP32R, tag="b_ada_sl", name=f"bada_{mb}")
        nc.sync.dma_start(out=b_sl, in_=io["b_ada"][:, 512 * mb:512 * (mb + 1)])
        pm = ps_aux.tile([1, 512], FP32, tag="aux")
        for k in range(4):
            nc.tensor.matmul(pm, cs[:, k:k + 1], wt[:, k, :], start=(k == 0), stop=False)
        nc.tensor.matmul(pm, ones[0:1, 0:1], b_sl, start=False, stop=True)
        mp = vecp.tile([1, 512], FP32, tag="tmpv", name=f"modp_{mb}")
        nc.scalar.copy(mp, pm)
        nc.sync.dma_start(out=ar_mod_in[:, 512 * mb:512 * (mb + 1)], in_=mp)
    # Split mod AllReduce: the msa groups (cols 0:2048) gate LN1 and reduce after
    # only 4 of 12 ada blocks; the rest reduces later, off the critical path.
    # (nocc-sim charges +4us for the extra DMA op but cannot see the collective
    # latency this hides on real HW.)
    modT = sb.tile([128, 6, 8], FP32, tag="modT")
    ksc = sb.tile([128, 3, 8], FP32, tag="ksc")
    mod_view = ar_mod_out.rearrange("o (g j p) -> (o p) g j", p=128, g=6)
    for (c0, c1, g0, g1) in [(0, 2048, 0, 2), (2048, 6144, 2, 6)]:
        if nocc:
            nc.sync.dma_start(out=ar_mod_out[:, c0:c1], in_=ar_mod_in[:, c0:c1])
        else:
            nc.gpsimd.collective_compute(
                "AllReduce", ALU.add, replica_groups=REPLICA_GROUPS,
                ins=[ar_mod_in[:, c0:c1].opt()], outs=[ar_mod_out[:, c0:c1].opt()])
        nc.sync.dma_start(out=modT[:, g0:g1, :], in_=mod_view[:, g0:g1, :])
        for i in range(g0 // 2, g1 // 2):
            nc.vector.tensor_scalar(ksc[:, i, :], modT[:, 2 * i + 1, :], 1.0, None,
                                    op0=ALU.add)
    eps_t = sb.tile([1, 1], FP32, tag="eps")
    nc.vector.memset(eps_t, EPS)

    # ---------------- LN + modulate ----------------
    def layer_norm_mod(x_tiles, g_sh, g_sc):
        sum_ps = ps_aux.tile([1, T], FP32, tag="aux")
        sq_ps = ps_aux.tile([1, T], FP32, tag="aux")
        for j in range(NCH):
            xsq = xsqp.tile([128, T], FP32R, tag="xsq")
            nc.vector.tensor_tensor(xsq, x_tiles[j], x_tiles[j], op=ALU.mult)
            for nb in range(2):
                s = slice(512 * nb, 512 * (nb + 1))
                nc.tensor.matmul(sum_ps[:, s], ones[:, 0:1], x_tiles[j][:, s],
                                 start=(j == 0), stop=(j == NCH - 1), skip_group_check=True)
                nc.tensor.matmul(sq_ps[:, s], ones[:, 1:2], xsq[:, s],
                                 start=(j == 0), stop=(j == NCH - 1), skip_group_check=True)
        mu = vecp.tile([1, T], FP32R, tag="mu")
        nc.scalar.activation(mu, sum_ps, AF.Copy, scale=1.0 / D)
        musq = vecp.tile([1, T], FP32, tag="tmpv")
        nc.vector.tensor_tensor(musq, mu, mu, op=ALU.mult)
        var = vecp.tile([1, T], FP32, tag="tmpv2")
        nc.vector.scalar_tensor_tensor(var, sq_ps, 1.0 / D, musq,
                                       op0=ALU.mult, op1=ALU.subtract)
        sig = vecp.tile([1, T], FP32, tag="tmpv")
        nc.scalar.activation(sig, var, AF.Sqrt, bias=eps_t)
        rsig = vecp.tile([1, T], FP32R, tag="rsig")
        with nc.allow_low_precision(reason="fp32r rsig feeds fp32r broadcast matmul"):
            nc.vector.reciprocal(rsig, sig)
        mu_b = ps_aux.tile([128, T], FP32, tag="aux")
        rs_b = ps_aux.tile([128, T], FP32, tag="aux")
        for nb in range(2):
            s = slice(512 * nb, 512 * (nb + 1))
            nc.tensor.matmul(mu_b[:, s], ones[0:1, :], mu[:, s], start=True, stop=True)
            nc.tensor.matmul(rs_b[:, s], ones[0:1, :], rsig[:, s], start=True, stop=True)
        h_tiles = []
        for j in range(NCH):
            t1 = scr.tile([128, T], FP32, tag="t1")
            nc.vector.tensor_tensor(t1, x_tiles[j], mu_b, op=ALU.subtract)
            nc.vector.scalar_tensor_tensor(t1, t1, ksc[:, g_sc, j:j + 1], rs_b,
                                           op0=ALU.mult, op1=ALU.mult)
            h = hp.tile([128, T], FP32R, tag="h")
            nc.vector.tensor_scalar(h, t1, modT[:, g_sh, j:j + 1], None, op0=ALU.add)
            h_tiles.append(h)
        return h_tiles

    # ---------------- Stage 1: x + LN1 ----------------
    x1 = []
    for j in range(NCH):
        xt = xp.tile([128, T], FP32R, tag="x")
        nc.sync.dma_start(out=xt, in_=io["xT"][128 * j:128 * (j + 1), :])
        x1.append(xt)
    h1 = layer_norm_mod(x1, g_sh=0, g_sc=0)

    # ---------------- Stage 2: qkv + vT ----------------
    b_qkT = sb.tile([128, 8], FP32, tag="b_qkT")
    nc.sync.dma_start(out=b_qkT, in_=io["b_qkT"])
    bv_b = sb.tile([128, DL], FP32, tag="bv")
    bv_src = io["b_v"]
    nc.sync.dma_start(out=bv_b, in_=bass.AP(tensor=bv_src.tensor, offset=bv_src.offset,
                                            ap=[[0, 128]] + list(bv_src.ap[1:])))
    # q,k: feature-major out, 8 m-chunks (0..3 q, 4..7 k) via 4 M-blocks of 256
    qkT = []
    for mb in range(4):
        wt = load_wblock(io["w_qkv"], FP32R, mb, wst)
        for mm in range(2):
            m = 2 * mb + mm
            pm = ps_mm.tile([128, T], FP32, tag="mm")
            for nb in range(2):
                s = slice(512 * nb, 512 * (nb + 1))
                for k in range(NCH):
                    nc.tensor.matmul(pm[:, s], wt[:, k, 128 * mm:128 * (mm + 1)],
                                     h1[k][:, s], start=(k == 0), stop=(k == NCH - 1))
            qk = qkp.tile([128, T], BF16, tag="qk")
            nc.vector.tensor_scalar(qk, pm, b_qkT[:, m:m + 1], None, op0=ALU.add)
            qkT.append(qk)
    # vT: token-major out with ones-augmented head columns
    wv_blk = [load_wblock(io["w_qkv"], FP32R, 4 + g, wst)
              for g in range(2)]
    vT = []
    for m in range(NCH):
        pm = ps_aux.tile([128, DL], FP32, tag="aux")
        for g in range(2):
            for k in range(NCH):
                nc.tensor.matmul(pm[:, 256 * g:256 * (g + 1)],
                                 h1[k][:, 128 * m:128 * (m + 1)], wv_blk[g][:, k, :],
                                 start=(k == 0), stop=(k == NCH - 1))
        vt = vtp.tile([128, HL, HD + 1], BF16, tag="vt")
        nc.vector.tensor_tensor(vt[:, :, 0:HD],
                                pm.rearrange("p (a b) -> p a b", a=HL),
                                bv_b.rearrange("p (a b) -> p a b", a=HL), op=ALU.add)
        nc.sync.dma_start(out=vt[:, :, HD:HD + 1],
                          in_=io["ones_b"].rearrange("p (j o) -> p j o", o=1))
        vT.append(vt)

    # ---------------- attention ----------------
    def attention(q_tiles, k_tiles, v_tiles, ktoks):
        nchunk = (ktoks + 127) // 128
        at_tiles = [atp.tile([128, T], BF16, tag="at", name=f"at_{i}") for i in range(4)]
        for h in range(HL):
            ti, off = h // 2, 64 * (h % 2)
            q_ap = q_tiles[ti][off:off + 64, :]
            pts = []
            for m in range(nchunk):
                mk = min(128, ktoks - 128 * m)
                ps_sc = ps_mm.tile([128, T], FP32, tag="mm")
                k_ap = k_tiles[ti][off:off + 64, 128 * m:128 * m + mk]
                for nb in range(2):
                    s = slice(512 * nb, 512 * (nb + 1))
                    nc.tensor.matmul(ps_sc[:mk, s], k_ap, q_ap[:, s], start=True, stop=True)
                pt = pp.tile([128, T], BF16, tag="p")
                nc.scalar.activation(pt[:mk, :], ps_sc[:mk, :], AF.Exp,
                                     scale=float(HD) ** -0.5)
                pts.append((pt, mk))
            po = ps_aux.tile([65, T], FP32, tag="aux")
            for m, (pt, mk) in enumerate(pts):
                for nb in range(2):
                    s = slice(512 * nb, 512 * (nb + 1))
                    nc.tensor.matmul(po[:, s], v_tiles[m][:mk, h, :], pt[:mk, s],
                                     start=(m == 0), stop=(m == nchunk - 1),
                                     skip_group_check=True)
            rcp = rcpp.tile([1, T], FP32R, tag="rcp")
            with nc.allow_low_precision(reason="fp32r rcp feeds fp32r broadcast matmul"):
                nc.vector.reciprocal(rcp, po[64:65, :])
            pb = ps_aux.tile([64, T], FP32, tag="aux")
            for nb in range(2):
                s = slice(512 * nb, 512 * (nb + 1))
                nc.tensor.matmul(pb[:, s], ones[0:1, 0:64], rcp[:, s], start=True, stop=True)
            rc_sb = rcpp.tile([64, T], FP32, tag="rc_sb")
            nc.vector.tensor_copy(rc_sb, pb)
            nc.vector.tensor_tensor(at_tiles[ti][off:off + 64, :], po[0:64, :], rc_sb,
                                    op=ALU.mult)
        return at_tiles

    at1 = attention(qkT[0:4], qkT[4:8], vT, T)

    # ---------------- row-parallel + fold residual + AllReduce ----------------
    def row_parallel_reduce(w_name, bT_name, act_tiles, x_tiles, nk, ar_tag):
        bT = sb.tile([128, 8], FP32, tag=bT_name)
        nc.sync.dma_start(out=bT, in_=io[bT_name])
        ar_in = dram.tile([D, T], FP32, tag=ar_tag + "_i")
        ar_out = dram.tile([D, T], FP32, tag=ar_tag + "_o")
        for mb in range(2):
            wt = load_wblock(io[w_name], BF16, mb, wst2)
            for mm in range(4):
                m = 4 * mb + mm
                pm = ps_mm.tile([128, T], FP32, tag="mm")
                for nb in range(2):
                    s = slice(512 * nb, 512 * (nb + 1))
                    for k in range(nk):
                        nc.tensor.matmul(pm[:, s], wt[:, k, 128 * mm:128 * (mm + 1)],
                                         act_tiles[k][:, s], start=(k == 0),
                                         stop=(k == nk - 1))
                e2 = scr.tile([128, T], FP32, tag="t1")
                nc.vector.tensor_scalar(e2, pm, bT[:, m:m + 1], None, op0=ALU.add)
                nc.vector.scalar_tensor_tensor(e2, x_tiles[m], mask, e2,
                                               op0=ALU.mult, op1=ALU.add)
                nc.sync.dma_start(out=ar_in[128 * m:128 * (m + 1), :], in_=e2)
        for hh in range(2):
            sl = slice(hh * D // 2, (hh + 1) * D // 2)
            if nocc:
                nc.sync.dma_start(out=ar_out[sl, :], in_=ar_in[sl, :])
            else:
                nc.gpsimd.collective_compute(
                    "AllReduce", ALU.add, replica_groups=REPLICA_GROUPS,
                    ins=[ar_in[sl, :].opt()], outs=[ar_out[sl, :].opt()])
        newx = []
        for m in range(NCH):
            xt = xp.tile([128, T], FP32R, tag="x")
            nc.gpsimd.dma_start(out=xt, in_=ar_out[128 * m:128 * (m + 1), :])
            newx.append(xt)
        return newx

    x2 = row_parallel_reduce("w_proj", "b_projT", at1, x1, 4, "arp")

    # ---------------- Stage 4: cross-attention ----------------
    teT = []
    for k in range(TD // 128):
        tt = pools["tep"].tile([128, TLP], FP32R, tag="teT")
        nc.sync.dma_start(out=tt, in_=io["teT"][128 * k:128 * (k + 1), :])
        teT.append(tt)
    b_ctxT = sb.tile([128, 8], FP32, tag="b_ctxT")
    nc.sync.dma_start(out=b_ctxT, in_=io["b_ctxT"])
    ctxT = []
    for mb in range(4):
        wt = load_wblock(io["w_ctx"], FP32R, mb, wst)
        for mm in range(2):
            m = 2 * mb + mm
            pm = ps_aux.tile([128, TLP], FP32, tag="aux")
            for k in range(TD // 128):
                nc.tensor.matmul(pm, wt[:, k, 128 * mm:128 * (mm + 1)], teT[k],
                                 start=(k == 0), stop=(k == TD // 128 - 1))
            ct = pools["ctxp"].tile([128, TLP], FP32R, tag="ctxT")
            nc.vector.tensor_scalar(ct, pm, b_ctxT[:, m:m + 1], None, op0=ALU.add)
            ctxT.append(ct)

    h2 = layer_norm_mod(x2, g_sh=2, g_sc=1)

    q2T = []
    for mb in range(2):
        wt = load_wblock(io["w_q"], FP32R, mb, wst)
        for mm in range(2):
            m = 2 * mb + mm
            pm = ps_mm.tile([128, T], FP32, tag="mm")
            for nb in range(2):
                s = slice(512 * nb, 512 * (nb + 1))
                for k in range(NCH):
                    nc.tensor.matmul(pm[:, s], wt[:, k, 128 * mm:128 * (mm + 1)],
                                     h2[k][:, s], start=(k == 0), stop=(k == NCH - 1))
            qt = qkp.tile([128, T], BF16, tag="qk")
            nc.vector.tensor_copy(qt, pm)
            q2T.append(qt)
    kcT = []
    for mb in range(2):
        wt = load_wblock(io["w_k"], FP32R, mb, wst)
        for mm in range(2):
            m = 2 * mb + mm
            pm = ps_aux.tile([128, TLP], FP32, tag="aux")
            for k in range(NCH):
                nc.tensor.matmul(pm, wt[:, k, 128 * mm:128 * (mm + 1)], ctxT[k],
                                 start=(k == 0), stop=(k == NCH - 1))
            kt = qkp.tile([128, T], BF16, tag="qk")
            nc.vector.tensor_copy(kt[:, 0:TLP], pm)
            kcT.append(kt)
    # vc: [77, HL, 65]
    vc_blk = [load_wblock(io["w_v"], FP32R, g, wst) for g in range(2)]
    pv = ps_aux.tile([TLP, DL], FP32, tag="aux")
    for g in range(2):
        for k in range(NCH):
            nc.tensor.matmul(pv[:, 256 * g:256 * (g + 1)], ctxT[k],
                             vc_blk[g][:, k, :], start=(k == 0), stop=(k == NCH - 1))
    vc = vtp.tile([128, HL, HD + 1], BF16, tag="vt")
    nc.vector.memset(vc, 0.0)
    nc.vector.tensor_copy(vc[0:TL, :, 0:HD], pv[0:TL].rearrange("p (a b) -> p a b", a=HL))
    nc.sync.dma_start(out=vc[0:TL, :, HD:HD + 1],
                      in_=io["ones_b"][0:TL, :].rearrange("p (j o) -> p j o", o=1))

    at2 = attention(q2T, kcT, [vc], TLP)
    x3 = row_parallel_reduce("w_out", "b_outT", at2, x2, 4, "aro")

    # ---------------- Stage 5: MLP ----------------
    h3 = layer_norm_mod(x3, g_sh=4, g_sc=2)
    b_fc1T = sb.tile([128, 16], FP32, tag="b_fc1T")
    nc.sync.dma_start(out=b_fc1T, in_=io["b_fc1T"])
    b_fc2T = sb.tile([128, 8], FP32, tag="b_fc2T")
    nc.sync.dma_start(out=b_fc2T, in_=io["b_fc2T"])
    for tb in range(2):
        s = slice(512 * tb, 512 * (tb + 1))
        hid = []
        for mb in range(8):
            wt = load_wblock(io["w_fc1"], FP32R, mb, wst)
            for mm in range(2):
                m = 2 * mb + mm
                pm = ps_aux.tile([128, 512], FP32, tag="aux")
                for k in range(NCH):
                    nc.tensor.matmul(pm, wt[:, k, 128 * mm:128 * (mm + 1)], h3[k][:, s],
                                     start=(k == 0), stop=(k == NCH - 1))
                ht = hidp.tile([128, 512], BF16, tag="hid")
                nc.scalar.activation(ht, pm, AF.Gelu, bias=b_fc1T[:, m:m + 1])
                hid.append(ht)
        for mb in range(4):
            wt2 = load_wblock(io["w_fc2"], BF16, mb, wst2)
            for mm in range(2):
                m = 2 * mb + mm
                pm = ps_mm.tile([128, 512], FP32, tag="mm")
                for k in range(FFL // 128):
                    nc.tensor.matmul(pm, wt2[:, k, 128 * mm:128 * (mm + 1)], hid[k],
                                     start=(k == 0), stop=(k == FFL // 128 - 1))
                ot = scr.tile([128, 512], FP32R, tag="ot")
                nc.vector.tensor_scalar(ot, pm, b_fc2T[:, m:m + 1], None, op0=ALU.add)
                nc.vector.scalar_tensor_tensor(ot, x3[m][:, s], mask, ot,
                                               op0=ALU.mult, op1=ALU.add)
                nc.sync.dma_start(out=io["out_xT"][128 * m:128 * (m + 1), s], in_=ot)


def build(nocc=False):
    nc = bacc.Bacc("TRN2", target_bir_lowering=False, debug=False,
                   num_devices=1 if nocc else 8)
    io = _declare(nc)
    with tile.TileContext(nc) as tc:
        import contextlib
        with contextlib.ExitStack() as ctx:
            def pool(name, bufs, space="SBUF"):
                return ctx.enter_context(tc.tile_pool(name=name, bufs=bufs, space=space))
            pools = {
                "sb": pool("sb", 1),
                "xp": pool("xp", 8),
                "hp": pool("hp", 8),
                "qkp": pool("qkp", 8),
                "vtp": pool("vtp", 8),
                "atp": pool("atp", 4),
                "pp": pool("pp", P_BUFS),
                "wst": pool("wst", WST_BUFS),
                "wst2": pool("wst2", WST2_BUFS),
                "hidp": pool("hidp", 16),
                "xsqp": pool("xsqp", XSQ_BUFS),
                "scr": pool("scr", SCR_BUFS),
                "rcpp": pool("rcpp", 1),
                "tep": pool("tep", 6),
                "ctxp": pool("ctxp", 8),
                "vecp": pool("vecp", 1),
                "ps_mm": pool("ps_mm", 2, "PSUM"),
                "ps_aux": pool("ps_aux", 2, "PSUM"),
                "dram": pool("dram", 1, "DRAM"),
            }
            _emit(tc, io, pools, nocc=nocc)
    nc.compile()
    return nc


def pretile(w, mblk):
    """[K, M] -> [M//mblk, 128, K//128, mblk] contiguous blocks."""
    K, M = w.shape
    kc = K // 128
    v = w.reshape(kc, 128, M // mblk, mblk).transpose(2, 1, 0, 3)
    return np.ascontiguousarray(v)


def shard_inputs(inputs):
    f32 = np.float32
    bf16 = ml_dtypes.bfloat16
    x = np.asarray(inputs["x"], f32)
    c = np.asarray(inputs["c"], f32)
    te = np.asarray(inputs["text_embed"], f32)
    W_ada, b_ada = np.asarray(inputs["W_ada"], f32), np.asarray(inputs["b_ada"], f32)
    W_qkv, b_qkv = np.asarray(inputs["W_qkv"], f32), np.asarray(inputs["b_qkv"], f32)
    W_proj, b_proj = np.asarray(inputs["W_proj"], f32), np.asarray(inputs["b_proj"], f32)
    W_ctx, b_ctx = np.asarray(inputs["W_ctx"], f32), np.asarray(inputs["b_ctx"], f32)
    W_q, W_k, W_v = (np.asarray(inputs[k], f32) for k in ("W_q", "W_k", "W_v"))
    W_out, b_out = np.asarray(inputs["W_out"], f32), np.asarray(inputs["b_out"], f32)
    W_fc1, b_fc1 = np.asarray(inputs["W_fc1"], f32), np.asarray(inputs["b_fc1"], f32)
    W_fc2, b_fc2 = np.asarray(inputs["W_fc2"], f32), np.asarray(inputs["b_fc2"], f32)

    maps = []
    for core in range(8):
        b, hf = core // 2, core % 2
        sl = slice(DL * hf, DL * (hf + 1))
        half = (lambda a: a) if hf == 0 else (lambda a: np.zeros_like(a))
        qs = slice(DL * hf, DL * (hf + 1))
        ks_ = slice(D + DL * hf, D + DL * (hf + 1))
        vs = slice(2 * D + DL * hf, 2 * D + DL * (hf + 1))
        m = {
            "xT": np.ascontiguousarray(x[b].T),
            "cT": np.ascontiguousarray(c[b, sl].reshape(4, 128).T),
            "teT": np.ascontiguousarray(np.pad(te[b].T, ((0, 0), (0, TLP - TL)))),
            "w_ada": pretile(W_ada[sl, :], 512),
            "b_ada": half(b_ada)[None, :],
            "w_qkv": pretile(np.concatenate(
                [W_qkv[:, qs], W_qkv[:, ks_], W_qkv[:, vs]], axis=1), 256),
            "b_qkT": np.ascontiguousarray(
                np.concatenate([b_qkv[qs], b_qkv[ks_]]).reshape(8, 128).T),
            "b_v": b_qkv[vs][None, :],
            "w_proj": pretile(W_proj[sl, :].astype(bf16), 512),
            "b_projT": np.ascontiguousarray(half(b_proj).reshape(8, 128).T),
            "w_ctx": pretile(W_ctx, 256),
            "b_ctxT": np.ascontiguousarray(b_ctx.reshape(8, 128).T),
            "w_q": pretile(W_q[:, sl], 256),
            "w_k": pretile(W_k[:, sl], 256),
            "w_v": pretile(W_v[:, sl], 256),
            "w_out": pretile(W_out[sl, :].astype(bf16), 512),
            "b_outT": np.ascontiguousarray(half(b_out).reshape(8, 128).T),
            "w_fc1": pretile(W_fc1[:, FFL * hf:FFL * (hf + 1)], 256),
            "b_fc1T": np.ascontiguousarray(
                b_fc1[FFL * hf:FFL * (hf + 1)].reshape(16, 128).T),
            "w_fc2": pretile(W_fc2[FFL * hf:FFL * (hf + 1), :].astype(bf16), 256),
            "b_fc2T": np.ascontiguousarray(half(b_fc2).reshape(8, 128).T),
            "ones_r": np.ones((128, 128), f32),
            "ones_b": np.ones((128, 8), bf16),
            "maskT": np.full((128, 1), 1.0 - hf, f32),
        }
        maps.append(m)
    return maps


_NC_CACHE = None


def kernel(**inputs):
    global _NC_CACHE
    if _NC_CACHE is None:
        _NC_CACHE = build()
    nc = _NC_CACHE
    in_maps = shard_inputs(inputs)
    res = run_bass_kernel_spmd(nc, in_maps, core_ids=list(range(8)))
    out = np.empty((B, N, D), np.float32)
    for b in range(B):
        p0 = res.results[2 * b]["out_xT"]
        p1 = res.results[2 * b + 1]["out_xT"]
        out[b] = (p0.astype(np.float32) + p1.astype(np.float32)).T
    return out



# revision 22
# speedup vs baseline: 1.3849x; 1.3849x over previous
"""DiT block kernel for TRN2, 8 NeuronCores.

Sharding: DP=4 over batch x TP=2 over heads (Megatron style).
Core c: batch b=c//2, half hf=c%2 (8 of 16 heads, 2048/4096 MLP cols, 512/1024
rows of the row-parallel weights).

Device layout is feature-major: activations [feature(partitions), token(free)], so
every matmul chains without transposes: outT = W.T @ actT with lhsT=W natural layout.
v^T (lhsT of attn@v) is produced by swapping matmul operands. Softmax runs without
max-subtraction (scores are O(1)); denominators come from an augmented ones-column in
v^T (row 64 of the [65, tok] attention output).

Dtypes: bf16 trunk (weights, activations, residuals, collective payloads); PSUM fp32;
the MLP (fc1/fc2) runs in fp8e4m3 with DoubleRow perf mode (weights pre-scaled x32 on
host, activations cast to fp8 in the LN3 modulate / gelu ops; descale folded into the
activation-scale and output tensor_scalar).

Comms: 3 pair AllReduces (adaLN mod, self-attn proj+residual, cross-attn out+residual);
the fc2 reduce is folded into the output (host adds the two partials).
"""
import sys
import numpy as np

sys.path.insert(0, "/opt/trn_rl_repo")

import ml_dtypes
import concourse.bass as bass
import concourse.mybir as mybir
import concourse.tile as tile
from concourse import bacc
from concourse.bass_utils import run_bass_kernel_spmd

FP32 = mybir.dt.float32
BF16 = mybir.dt.bfloat16
FP8 = mybir.dt.float8e4
AF = mybir.ActivationFunctionType
ALU = mybir.AluOpType
DR = mybir.MatmulPerfMode.DoubleRow

B, N, D, H, TD, TL = 4, 1024, 1024, 16, 768, 77
HD = 64
EPS = 1e-6
HL = 8
DL = 512
FFL = 2048
T = 1024
TLP = 80    # ctx tokens padded
NCH = D // 128
REPLICA_GROUPS = [[0, 1], [2, 3], [4, 5], [6, 7]]

W8_SCALE = 32.0     # fp8 weight pre-scale (host)
X8_SCALE = 16.0     # fp8 h3 activation scale (kernel)

# SBUF knobs
P_BUFS = 7
WST_BUFS = 3


def _declare(nc):
    d = {}

    def inp(name, shape, dt):
        d[name] = nc.dram_tensor(name, list(shape), dt, kind="ExternalInput").ap()

    inp("xT", (D, T), BF16)
    inp("cT", (128, 4), FP32)
    inp("teT", (TD, TLP), BF16)
    inp("w_ada", (12, 128, 4, 512), BF16)
    inp("b_ada", (1, 6 * D), BF16)
    inp("w_qkv", (6, 128, 8, 256), BF16)
    inp("b_qkT", (128, 8), FP32)
    inp("b_v", (1, DL), BF16)
    inp("w_proj", (2, 128, 4, 512), BF16)
    inp("b_projT", (128, 8), FP32)
    inp("w_ctx", (4, 128, 6, 256), BF16)
    inp("b_ctxT", (128, 8), FP32)
    inp("w_q", (2, 128, 8, 256), BF16)
    inp("w_k", (2, 128, 8, 256), BF16)
    inp("w_v", (2, 128, 8, 256), BF16)
    inp("w_out", (2, 128, 4, 512), BF16)
    inp("b_outT", (128, 8), FP32)
    inp("w_fc1", (8, 128, 8, 256), BF16)
    inp("b_fc1T", (128, 16), FP32)
    inp("w_fc2_hi", (4, 128, 8, 2, 256), FP8)
    inp("w_fc2_lo", (4, 128, 8, 2, 256), FP8)
    inp("b_fc2T", (128, 8), FP32)
    inp("ones_b", (128, 128), BF16)
    inp("maskT", (128, 1), FP32)
    d["out_xT"] = nc.dram_tensor("out_xT", [D, T], BF16, kind="ExternalOutput").ap()
    return d


def _emit(tc, io, pools, nocc=False):
    nc = tc.nc
    sb = pools["sb"]
    xp, hp, qkp, vtp, atp, pp = (pools[k] for k in ("xp", "hp", "qkp", "vtp", "atp", "pp"))
    wst, hidp, xsqp, scr, rcpp = (pools[k] for k in ("wst", "hidp", "xsqp", "scr", "rcpp"))
    vecp, bcp = pools["vecp"], pools["bcp"]
    ps_mm, ps_aux = pools["ps_mm"], pools["ps_aux"]
    dram = pools["dram"]

    ones = sb.tile([128, 128], BF16, tag="ones")
    nc.sync.dma_start(out=ones, in_=io["ones_b"])
    mask = sb.tile([128, 1], FP32, tag="mask")
    nc.sync.dma_start(out=mask, in_=io["maskT"])

    def load_wblock(src_ap, dt, bi, pool, engine=None):
        """Load pre-tiled weight block bi -> one contiguous-per-partition DMA."""
        shp = list(src_ap.shape[1:])
        wt = pool.tile([128] + shp[1:], dt, tag="w")
        (engine or nc.sync).dma_start(out=wt, in_=src_ap[bi])
        return wt

    # ---------------- Stage 1a: x load (first: feeds LN1 stats + residual) ----
    x1 = []
    for j in range(NCH):
        xt = xp.tile([128, T], BF16, tag="x")
        nc.sync.dma_start(out=xt, in_=io["xT"][128 * j:128 * (j + 1), :])
        x1.append(xt)

    # ---------------- LN stats (PE sums; DVE squares) ----------------
    def ln_stats(x_tiles):
        """Returns ([1,T] sum(x), [1,T] sum(x^2)) PSUM tiles."""
        sum_ps = ps_aux.tile([1, T], FP32, tag="aux")
        sq_ps = ps_aux.tile([1, T], FP32, tag="aux")
        for j in range(NCH):
            xsq = xsqp.tile([128, T], BF16, tag="xsq")
            nc.vector.tensor_tensor(xsq, x_tiles[j], x_tiles[j], op=ALU.mult)
            for nb in range(2):
                s = slice(512 * nb, 512 * (nb + 1))
                nc.tensor.matmul(sum_ps[:, s], ones[:, 0:1], x_tiles[j][:, s],
                                 start=(j == 0), stop=(j == NCH - 1), skip_group_check=True)
                nc.tensor.matmul(sq_ps[:, s], ones[:, 1:2], xsq[:, s],
                                 start=(j == 0), stop=(j == NCH - 1), skip_group_check=True)
        return sum_ps, sq_ps

    st1 = ln_stats(x1)

    # ---------------- Stage 0: adaLN ----------------
    cT = sb.tile([128, 4], FP32, tag="cT")
    nc.sync.dma_start(out=cT, in_=io["cT"])
    cs = sb.tile([128, 4], BF16, tag="cs")
    nc.scalar.activation(cs, cT, AF.Silu)

    ar_mod_in = dram.tile([1, 6 * D], BF16, tag="armod_i")
    ar_mod_out = dram.tile([1, 6 * D], BF16, tag="armod_o")

    b_ada_sb = sb.tile([1, 6 * D], BF16, tag="b_ada_sb")
    nc.sync.dma_start(out=b_ada_sb, in_=io["b_ada"])
    mod_sb = sb.tile([1, 6 * D], BF16, tag="mod_sb")
    for mb in range(12):   # w_ada rhs-blocks [512, 512]
        wt = load_wblock(io["w_ada"], BF16, mb, wst)
        pm = ps_mm.tile([1, 512], FP32, tag="mm")
        for k in range(4):
            nc.tensor.matmul(pm, cs[:, k:k + 1], wt[:, k, :], start=(k == 0), stop=False)
        nc.tensor.matmul(pm, ones[0:1, 0:1], b_ada_sb[:, 512 * mb:512 * (mb + 1)],
                         start=False, stop=True)
        if mb % 2 == 0:
            nc.scalar.copy(mod_sb[:, 512 * mb:512 * (mb + 1)], pm)
        else:
            nc.vector.tensor_copy(mod_sb[:, 512 * mb:512 * (mb + 1)], pm)
        if mb == 3:
            nc.sync.dma_start(out=ar_mod_in[:, 0:2048], in_=mod_sb[:, 0:2048])
        elif mb == 11:
            nc.sync.dma_start(out=ar_mod_in[:, 2048:6144], in_=mod_sb[:, 2048:6144])
    # Split mod AllReduce: msa groups (cols 0:2048) reduce after only 4 of 12
    # ada blocks to unblock LN1's modulate early.
    modTb = sb.tile([128, 6, 8], BF16, tag="modTb")
    modT = sb.tile([128, 6, 8], FP32, tag="modT")
    ksc = sb.tile([128, 3, 8], FP32, tag="ksc")
    mod_view = ar_mod_out.rearrange("o (g j p) -> (o p) g j", p=128, g=6)
    for (c0, c1, g0, g1) in [(0, 2048, 0, 2), (2048, 6144, 2, 6)]:
        if nocc:
            nc.sync.dma_start(out=ar_mod_out[:, c0:c1], in_=ar_mod_in[:, c0:c1])
        else:
            nc.gpsimd.collective_compute(
                "AllReduce", ALU.add, replica_groups=REPLICA_GROUPS,
                ins=[ar_mod_in[:, c0:c1].opt()], outs=[ar_mod_out[:, c0:c1].opt()])
        nc.sync.dma_start(out=modTb[:, g0:g1, :], in_=mod_view[:, g0:g1, :])
        nc.vector.tensor_scalar(modT[:, g0:g1, :], modTb[:, g0:g1, :], 0.0, None,
                                op0=ALU.add)
        for i in range(g0 // 2, g1 // 2):
            nc.vector.tensor_scalar(ksc[:, i, :], modT[:, 2 * i + 1, :], 1.0, None,
                                    op0=ALU.add)
    eps_t = sb.tile([1, 1], FP32, tag="eps")
    nc.vector.memset(eps_t, EPS)

    # ---------------- LN + modulate ----------------
    def layer_norm_mod(stats, x_tiles, g_sh, g_sc, outs=None, kv=None):
        """kv: (ksc_ap, sh_ap) [128, 8] scalar tables overriding modT groups.
        outs: optional per-chunk output APs (e.g. fp8 slices of one tile)."""
        sum_ps, sq_ps = stats
        mu = vecp.tile([1, T], BF16, tag="mu")
        nc.scalar.activation(mu, sum_ps, AF.Copy, scale=1.0 / D)
        musq = vecp.tile([1, T], FP32, tag="tmpv")
        nc.vector.tensor_tensor(musq, mu, mu, op=ALU.mult)
        var = vecp.tile([1, T], FP32, tag="tmpv2")
        nc.vector.scalar_tensor_tensor(var, sq_ps, 1.0 / D, musq,
                                       op0=ALU.mult, op1=ALU.subtract)
        sig = vecp.tile([1, T], FP32, tag="tmpv")
        nc.scalar.activation(sig, var, AF.Sqrt, bias=eps_t)
        rsig = vecp.tile([1, T], BF16, tag="rsig")
        with nc.allow_low_precision(reason="bf16 rsig feeds bf16 broadcast matmul"):
            nc.vector.reciprocal(rsig, sig)
        mu_ps = ps_aux.tile([128, T], FP32, tag="aux")
        rs_ps = ps_aux.tile([128, T], FP32, tag="aux")
        for nb in range(2):
            s = slice(512 * nb, 512 * (nb + 1))
            nc.tensor.matmul(mu_ps[:, s], ones[0:1, :], mu[:, s], start=True, stop=True)
            nc.tensor.matmul(rs_ps[:, s], ones[0:1, :], rsig[:, s], start=True, stop=True)
        mu_sb = bcp.tile([128, T], BF16, tag="mu_sb")
        nc.scalar.copy(mu_sb, mu_ps)
        rs_sb = bcp.tile([128, T], BF16, tag="rs_sb")
        nc.vector.tensor_copy(rs_sb, rs_ps)
        kv = kv or (ksc[:, g_sc, :], modT[:, g_sh, :])
        h_tiles = []
        for j in range(NCH):
            t1 = scr.tile([128, T], BF16, tag="t1")
            nc.vector.tensor_tensor(t1, x_tiles[j], mu_sb, op=ALU.subtract)
            nc.vector.tensor_tensor(t1, t1, rs_sb, op=ALU.mult)
            h = outs[j] if outs is not None else hp.tile([128, T], BF16, tag="h")
            nc.vector.tensor_scalar(h, t1, kv[0][:, j:j + 1], kv[1][:, j:j + 1],
                                    op0=ALU.mult, op1=ALU.add)
            h_tiles.append(h)
        return h_tiles

    # ---------------- Stage 1b: LN1 modulate ----------------
    h1 = layer_norm_mod(st1, x1, g_sh=0, g_sc=0)

    # ---------------- Stage 2: qkv + vT ----------------
    b_qkT = sb.tile([128, 8], FP32, tag="b_qkT")
    nc.sync.dma_start(out=b_qkT, in_=io["b_qkT"])
    bv_b = sb.tile([128, DL], BF16, tag="bv")
    bv_src = io["b_v"]
    nc.sync.dma_start(out=bv_b, in_=bass.AP(tensor=bv_src.tensor, offset=bv_src.offset,
                                            ap=[[0, 128]] + list(bv_src.ap[1:])))
    # q,k: feature-major out, 8 m-chunks (0..3 q, 4..7 k) via 4 M-blocks of 256
    qkT = []
    for mb in range(4):
        wt = load_wblock(io["w_qkv"], BF16, mb, wst)
        for mm in range(2):
            m = 2 * mb + mm
            pm = ps_mm.tile([128, T], FP32, tag="mm")
            for nb in range(2):
                s = slice(512 * nb, 512 * (nb + 1))
                for k in range(NCH):
                    nc.tensor.matmul(pm[:, s], wt[:, k, 128 * mm:128 * (mm + 1)],
                                     h1[k][:, s], start=(k == 0), stop=(k == NCH - 1))
            qk = qkp.tile([128, T], BF16, tag="qk")
            nc.scalar.activation(qk, pm, AF.Identity, bias=b_qkT[:, m:m + 1])
            qkT.append(qk)
    # vT: token-major out with ones-augmented head columns; fp8, chunk-PAIRED
    # along the ktok dim for DoubleRow P@V.
    wv_blk = [load_wblock(io["w_qkv"], BF16, 4 + g, wst) for g in range(2)]
    vT = []
    for m in range(NCH):
        pm = ps_aux.tile([128, DL], FP32, tag="aux")
        for g in range(2):
            for k in range(NCH):
                nc.tensor.matmul(pm[:, 256 * g:256 * (g + 1)],
                                 h1[k][:, 128 * m:128 * (m + 1)], wv_blk[g][:, k, :],
                                 start=(k == 0), stop=(k == NCH - 1))
        if m % 2 == 0:
            vt = vtp.tile([128, 2, HL, HD + 1], FP8, tag="vt", name=f"vt_{m // 2}", bufs=4)
            vT.append(vt)
        i = m % 2
        nc.vector.tensor_tensor(vT[-1][:, i, :, 0:HD],
                                pm.rearrange("p (a b) -> p a b", a=HL),
                                bv_b.rearrange("p (a b) -> p a b", a=HL), op=ALU.add)
        nc.gpsimd.memset(vT[-1][:, i, :, HD:HD + 1], 1.0)

    # ---------------- attention ----------------
    def attention(q_tiles, k_tiles, v_tiles, ktoks, pv8=False):
        """pv8: v_tiles are fp8 [128, 2, HL, 65] ktok-chunk pairs; P goes fp8
        and P@V uses DoubleRow. Else v_tiles are bf16 [128, HL, 65]."""
        nchunk = (ktoks + 127) // 128
        at_tiles = [atp.tile([128, T], BF16, tag="at", name=f"at_{i}") for i in range(4)]
        for h in range(HL):
            ti, off = h // 2, 64 * (h % 2)
            q_ap = q_tiles[ti][off:off + 64, :]
            pts = []
            for m in range(nchunk):
                mk = min(128, ktoks - 128 * m)
                ps_sc = ps_mm.tile([128, T], FP32, tag="mm")
                k_ap = k_tiles[ti][off:off + 64, 128 * m:128 * m + mk]
                for nb in range(2):
                    s = slice(512 * nb, 512 * (nb + 1))
                    nc.tensor.matmul(ps_sc[:mk, s], k_ap, q_ap[:, s], start=True, stop=True)
                if pv8:
                    if m % 2 == 0:
                        pt = pp.tile([128, 2, T], FP8, tag="p", name=f"pt_{h}_{m // 2}")
                        pts.append(pt)
                    nc.scalar.activation(pts[-1][:, m % 2, :], ps_sc, AF.Exp,
                                         scale=float(HD) ** -0.5)
                else:
                    pt = pp.tile([128, T], BF16, tag="px", bufs=2)
                    nc.scalar.activation(pt[:mk, :], ps_sc[:mk, :], AF.Exp,
                                         scale=float(HD) ** -0.5)
                    pts.append((pt, mk))
            po = ps_aux.tile([65, T], FP32, tag="aux")
            if pv8:
                for mp_ in range(nchunk // 2):
                    for nb in range(2):
                        s = slice(512 * nb, 512 * (nb + 1))
                        nc.tensor.matmul(po[:, s], v_tiles[mp_][:, :, h, :],
                                         pts[mp_][:, :, s], perf_mode=DR,
                                         start=(mp_ == 0), stop=(mp_ == nchunk // 2 - 1),
                                         skip_group_check=True)
            else:
                for m, (pt, mk) in enumerate(pts):
                    for nb in range(2):
                        s = slice(512 * nb, 512 * (nb + 1))
                        nc.tensor.matmul(po[:, s], v_tiles[m][:mk, h, :], pt[:mk, s],
                                         start=(m == 0), stop=(m == nchunk - 1),
                                         skip_group_check=True)
            rcp = rcpp.tile([1, T], BF16, tag="rcp")
            with nc.allow_low_precision(reason="bf16 rcp feeds bf16 broadcast matmul"):
                nc.vector.reciprocal(rcp, po[64:65, :])
            pb = ps_aux.tile([64, T], FP32, tag="aux")
            for nb in range(2):
                s = slice(512 * nb, 512 * (nb + 1))
                nc.tensor.matmul(pb[:, s], ones[0:1, 0:64], rcp[:, s], start=True, stop=True)
            rc_sb = rcpp.tile([64, T], BF16, tag="rc_sb")
            nc.vector.tensor_copy(rc_sb, pb)
            nc.vector.tensor_tensor(at_tiles[ti][off:off + 64, :], po[0:64, :], rc_sb,
                                    op=ALU.mult)
        return at_tiles

    at1 = attention(qkT[0:4], qkT[4:8], vT, T, pv8=True)

    # ---------------- row-parallel + fold residual + AllReduce ----------------
    def row_parallel_reduce(w_name, bT_name, act_tiles, x_tiles, nk, ar_tag):
        bT = sb.tile([128, 8], FP32, tag=bT_name)
        nc.sync.dma_start(out=bT, in_=io[bT_name])
        ar_in = dram.tile([D, T], BF16, tag=ar_tag + "_i")
        ar_out = dram.tile([D, T], BF16, tag=ar_tag + "_o")
        for mb in range(2):
            wt = load_wblock(io[w_name], BF16, mb, wst)
            for mm in range(4):
                m = 4 * mb + mm
                pm = ps_mm.tile([128, T], FP32, tag="mm")
                for nb in range(2):
                    s = slice(512 * nb, 512 * (nb + 1))
                    for k in range(nk):
                        nc.tensor.matmul(pm[:, s], wt[:, k, 128 * mm:128 * (mm + 1)],
                                         act_tiles[k][:, s], start=(k == 0),
                                         stop=(k == nk - 1))
                e2 = scr.tile([128, T], BF16, tag="t1")
                nc.vector.scalar_tensor_tensor(e2, x_tiles[m], mask, pm,
                                               op0=ALU.mult, op1=ALU.add)
                nc.vector.tensor_scalar(e2, e2, bT[:, m:m + 1], None, op0=ALU.add)
                nc.sync.dma_start(out=ar_in[128 * m:128 * (m + 1), :], in_=e2)
        for hh in range(2):
            sl = slice(hh * D // 2, (hh + 1) * D // 2)
            if nocc:
                nc.sync.dma_start(out=ar_out[sl, :], in_=ar_in[sl, :])
            else:
                nc.gpsimd.collective_compute(
                    "AllReduce", ALU.add, replica_groups=REPLICA_GROUPS,
                    ins=[ar_in[sl, :].opt()], outs=[ar_out[sl, :].opt()])
        newx = []
        for m in range(NCH):
            xt = xp.tile([128, T], BF16, tag="x")
            nc.gpsimd.dma_start(out=xt, in_=ar_out[128 * m:128 * (m + 1), :])
            newx.append(xt)
        return newx

    x2 = row_parallel_reduce("w_proj", "b_projT", at1, x1, 4, "arp")

    # ---------------- Stage 4: cross-attention ----------------
    teT = []
    for k in range(TD // 128):
        tt = pools["tep"].tile([128, TLP], BF16, tag="teT")
        nc.sync.dma_start(out=tt, in_=io["teT"][128 * k:128 * (k + 1), :])
        teT.append(tt)
    b_ctxT = sb.tile([128, 8], FP32, tag="b_ctxT")
    nc.sync.dma_start(out=b_ctxT, in_=io["b_ctxT"])
    ctxT = []
    for mb in range(4):
        wt = load_wblock(io["w_ctx"], BF16, mb, wst)
        for mm in range(2):
            m = 2 * mb + mm
            pm = ps_aux.tile([128, TLP], FP32, tag="aux")
            for k in range(TD // 128):
                nc.tensor.matmul(pm, wt[:, k, 128 * mm:128 * (mm + 1)], teT[k],
                                 start=(k == 0), stop=(k == TD // 128 - 1))
            ct = pools["ctxp"].tile([128, TLP], BF16, tag="ctxT")
            nc.vector.tensor_scalar(ct, pm, b_ctxT[:, m:m + 1], None, op0=ALU.add)
            ctxT.append(ct)

    st2 = ln_stats(x2)
    h2 = layer_norm_mod(st2, x2, g_sh=2, g_sc=1)

    q2T = []
    for mb in range(2):
        wt = load_wblock(io["w_q"], BF16, mb, wst)
        for mm in range(2):
            m = 2 * mb + mm
            pm = ps_mm.tile([128, T], FP32, tag="mm")
            for nb in range(2):
                s = slice(512 * nb, 512 * (nb + 1))
                for k in range(NCH):
                    nc.tensor.matmul(pm[:, s], wt[:, k, 128 * mm:128 * (mm + 1)],
                                     h2[k][:, s], start=(k == 0), stop=(k == NCH - 1))
            qt = qkp.tile([128, T], BF16, tag="qk")
            nc.scalar.copy(qt, pm)
            q2T.append(qt)
    kcT = []
    for mb in range(2):
        wt = load_wblock(io["w_k"], BF16, mb, wst)
        for mm in range(2):
            m = 2 * mb + mm
            pm = ps_aux.tile([128, TLP], FP32, tag="aux")
            for k in range(NCH):
                nc.tensor.matmul(pm, wt[:, k, 128 * mm:128 * (mm + 1)], ctxT[k],
                                 start=(k == 0), stop=(k == NCH - 1))
            kt = qkp.tile([128, T], BF16, tag="qk")
            nc.vector.tensor_copy(kt[:, 0:TLP], pm)
            kcT.append(kt)
    # vc: [77, HL, 65]
    vc_blk = [load_wblock(io["w_v"], BF16, g, wst) for g in range(2)]
    pv = ps_aux.tile([TLP, DL], FP32, tag="aux")
    for g in range(2):
        for k in range(NCH):
            nc.tensor.matmul(pv[:, 256 * g:256 * (g + 1)], ctxT[k],
                             vc_blk[g][:, k, :], start=(k == 0), stop=(k == NCH - 1))
    vc = vtp.tile([128, HL, HD + 1], BF16, tag="vc", bufs=1)
    nc.vector.memset(vc, 0.0)
    nc.vector.tensor_copy(vc[0:TL, :, 0:HD], pv[0:TL].rearrange("p (a b) -> p a b", a=HL))
    nc.gpsimd.memset(vc[0:TL, :, HD:HD + 1], 1.0)

    at2 = attention(q2T, kcT, [vc], TLP)
    x3 = row_parallel_reduce("w_out", "b_outT", at2, x2, 4, "aro")

    # ---------------- Stage 5: MLP ----------------
    # fc1: bf16 streamed. fc2: fp8 DoubleRow with dual (hi+lo) weights —
    # hi at x32 scale against hid, lo at x256 against hid/8 — accumulated in
    # one PSUM group; both descale by 1/32.
    b_fc1T = sb.tile([128, 16], FP32, tag="b_fc1T")
    nc.sync.dma_start(out=b_fc1T, in_=io["b_fc1T"])
    b_fc2T = sb.tile([128, 8], FP32, tag="b_fc2T")
    nc.sync.dma_start(out=b_fc2T, in_=io["b_fc2T"])

    st3 = ln_stats(x3)
    h3 = layer_norm_mod(st3, x3, g_sh=4, g_sc=2)

    hid = hidp.tile([128, FFL // 128, T], FP8, tag="hid")
    hid8 = hidp.tile([128, FFL // 128, T], FP8, tag="hid8")
    for mb in range(8):
        wt = load_wblock(io["w_fc1"], BF16, mb, wst)
        for mm in range(2):
            m = 2 * mb + mm
            for nb in range(2):
                s = slice(512 * nb, 512 * (nb + 1))
                pm = ps_mm.tile([128, 512], FP32, tag="mm")
                for k in range(NCH):
                    nc.tensor.matmul(pm, wt[:, k, 128 * mm:128 * (mm + 1)], h3[k][:, s],
                                     start=(k == 0), stop=(k == NCH - 1))
                nc.scalar.activation(hid[:, m, s], pm, AF.Gelu,
                                     bias=b_fc1T[:, m:m + 1])
            nc.gpsimd.tensor_scalar(hid8[:, m, :], hid[:, m, :], 0.125, None,
                                    op0=ALU.mult)
    for mb in range(4):
        whi = load_wblock(io["w_fc2_hi"], FP8, mb, wst)
        wlo = load_wblock(io["w_fc2_lo"], FP8, mb, wst)
        for mm in range(2):
            m = 2 * mb + mm
            xmb = scr.tile([128, T], BF16, tag="t1")
            nc.vector.tensor_scalar(xmb, x3[m], mask, b_fc2T[:, m:m + 1],
                                    op0=ALU.mult, op1=ALU.add)
            for nb in range(2):
                s = slice(512 * nb, 512 * (nb + 1))
                pm = ps_mm.tile([128, 512], FP32, tag="mm")
                for kp in range(8):
                    nc.tensor.matmul(pm, whi[:, kp, :, 128 * mm:128 * (mm + 1)],
                                     hid[:, 2 * kp:2 * kp + 2, s], perf_mode=DR,
                                     start=(kp == 0), stop=False)
                for kp in range(8):
                    nc.tensor.matmul(pm, wlo[:, kp, :, 128 * mm:128 * (mm + 1)],
                                     hid8[:, 2 * kp:2 * kp + 2, s], perf_mode=DR,
                                     start=False, stop=(kp == 7))
                ot = scr.tile([128, 512], BF16, tag="ot")
                nc.vector.scalar_tensor_tensor(ot, pm, 1.0 / W8_SCALE, xmb[:, s],
                                               op0=ALU.mult, op1=ALU.add)
                nc.sync.dma_start(out=io["out_xT"][128 * m:128 * (m + 1), s], in_=ot)


def build(nocc=False):
    nc = bacc.Bacc("TRN2", target_bir_lowering=False, debug=False,
                   num_devices=1 if nocc else 8)
    io = _declare(nc)
    with tile.TileContext(nc) as tc:
        import contextlib
        with contextlib.ExitStack() as ctx:
            def pool(name, bufs, space="SBUF"):
                return ctx.enter_context(tc.tile_pool(name=name, bufs=bufs, space=space))
            pools = {
                "sb": pool("sb", 1),
                "xp": pool("xp", 8),
                "hp": pool("hp", 8),
                "qkp": pool("qkp", 8),
                "vtp": pool("vtp", 8),
                "atp": pool("atp", 4),
                "pp": pool("pp", P_BUFS),
                "wst": pool("wst", WST_BUFS),
                "hidp": pool("hidp", 1),
                "xsqp": pool("xsqp", 1),
                "scr": pool("scr", 2),
                "rcpp": pool("rcpp", 1),
                "tep": pool("tep", 6),
                "ctxp": pool("ctxp", 8),
                "vecp": pool("vecp", 1),
                "bcp": pool("bcp", 2),
                "ps_mm": pool("ps_mm", 2, "PSUM"),
                "ps_aux": pool("ps_aux", 2, "PSUM"),
                "dram": pool("dram", 1, "DRAM"),
            }
            _emit(tc, io, pools, nocc=nocc)
    nc.compile()
    return nc


def pretile(w, mblk):
    """[K, M] -> [M//mblk, 128, K//128, mblk] contiguous blocks."""
    K, M = w.shape
    kc = K // 128
    v = w.reshape(kc, 128, M // mblk, mblk).transpose(2, 1, 0, 3)
    return np.ascontiguousarray(v)


def pretile_dr(w, mblk):
    """[K, M] -> [M//mblk, 128, K//256, 2, mblk] DoubleRow blocks."""
    K, M = w.shape
    kp = K // 256
    v = w.reshape(kp, 2, 128, M // mblk, mblk).transpose(3, 2, 0, 1, 4)
    return np.ascontiguousarray(v)


def shard_inputs(inputs):
    f32 = np.float32
    bf16 = ml_dtypes.bfloat16
    fp8 = ml_dtypes.float8_e4m3
    x = np.asarray(inputs["x"], f32)
    c = np.asarray(inputs["c"], f32)
    te = np.asarray(inputs["text_embed"], f32)
    W_ada, b_ada = np.asarray(inputs["W_ada"], f32), np.asarray(inputs["b_ada"], f32)
    W_qkv, b_qkv = np.asarray(inputs["W_qkv"], f32), np.asarray(inputs["b_qkv"], f32)
    W_proj, b_proj = np.asarray(inputs["W_proj"], f32), np.asarray(inputs["b_proj"], f32)
    W_ctx, b_ctx = np.asarray(inputs["W_ctx"], f32), np.asarray(inputs["b_ctx"], f32)
    W_q, W_k, W_v = (np.asarray(inputs[k], f32) for k in ("W_q", "W_k", "W_v"))
    W_out, b_out = np.asarray(inputs["W_out"], f32), np.asarray(inputs["b_out"], f32)
    W_fc1, b_fc1 = np.asarray(inputs["W_fc1"], f32), np.asarray(inputs["b_fc1"], f32)
    W_fc2, b_fc2 = np.asarray(inputs["W_fc2"], f32), np.asarray(inputs["b_fc2"], f32)

    maps = []
    for core in range(8):
        b, hf = core // 2, core % 2
        # fc2 dual-fp8: hi at x32; lo = fp8(256*(W - hi/32)), applied to hid/8
        w2_slice = W_fc2[FFL * hf:FFL * (hf + 1), :]
        w2_hi = (w2_slice * W8_SCALE).astype(fp8)
        w2_lo = ((w2_slice - w2_hi.astype(f32) / W8_SCALE) * 256.0).astype(fp8)
        sl = slice(DL * hf, DL * (hf + 1))
        half = (lambda a: a) if hf == 0 else (lambda a: np.zeros_like(a))
        qs = slice(DL * hf, DL * (hf + 1))
        ks_ = slice(D + DL * hf, D + DL * (hf + 1))
        vs = slice(2 * D + DL * hf, 2 * D + DL * (hf + 1))
        m = {
            "xT": np.ascontiguousarray(x[b].T).astype(bf16),
            "cT": np.ascontiguousarray(c[b, sl].reshape(4, 128).T),
            "teT": np.ascontiguousarray(
                np.pad(te[b].T, ((0, 0), (0, TLP - TL)))).astype(bf16),
            "w_ada": pretile(W_ada[sl, :].astype(bf16), 512),
            "b_ada": half(b_ada)[None, :].astype(bf16),
            "w_qkv": pretile(np.concatenate(
                [W_qkv[:, qs], W_qkv[:, ks_], W_qkv[:, vs]], axis=1).astype(bf16), 256),
            "b_qkT": np.ascontiguousarray(
                np.concatenate([b_qkv[qs], b_qkv[ks_]]).reshape(8, 128).T),
            "b_v": b_qkv[vs][None, :].astype(bf16),
            "w_proj": pretile(W_proj[sl, :].astype(bf16), 512),
            "b_projT": np.ascontiguousarray(half(b_proj).reshape(8, 128).T),
            "w_ctx": pretile(W_ctx.astype(bf16), 256),
            "b_ctxT": np.ascontiguousarray(b_ctx.reshape(8, 128).T),
            "w_q": pretile(W_q[:, sl].astype(bf16), 256),
            "w_k": pretile(W_k[:, sl].astype(bf16), 256),
            "w_v": pretile(W_v[:, sl].astype(bf16), 256),
            "w_out": pretile(W_out[sl, :].astype(bf16), 512),
            "b_outT": np.ascontiguousarray(half(b_out).reshape(8, 128).T),
            "w_fc1": pretile(W_fc1[:, FFL * hf:FFL * (hf + 1)].astype(bf16), 256),
            "b_fc1T": np.ascontiguousarray(
                b_fc1[FFL * hf:FFL * (hf + 1)].reshape(16, 128).T),
            "w_fc2_hi": pretile_dr(w2_hi, 256),
            "w_fc2_lo": pretile_dr(w2_lo, 256),
            "b_fc2T": np.ascontiguousarray(half(b_fc2).reshape(8, 128).T),
            "ones_b": np.ones((128, 128), bf16),
            "maskT": np.full((128, 1), 1.0 - hf, f32),
        }
        maps.append(m)
    return maps


_NC_CACHE = None


def kernel(**inputs):
    global _NC_CACHE
    if _NC_CACHE is None:
        _NC_CACHE = build()
    nc = _NC_CACHE
    in_maps = shard_inputs(inputs)
    res = run_bass_kernel_spmd(nc, in_maps, core_ids=list(range(8)))
    out = np.empty((B, N, D), np.float32)
    for b in range(B):
        p0 = res.results[2 * b]["out_xT"]
        p1 = res.results[2 * b + 1]["out_xT"]
        out[b] = (p0.astype(np.float32) + p1.astype(np.float32)).T
    return out
